# revision 4
# baseline (speedup 1.0000x reference)
"""Multi-head attention (B=4, S=2048, D=1024, H=16, Dh=64) on 8 trn2 cores.

Sharding: core c -> batch b=c//2, head-group g=c%2 (8 heads = 512 qkv cols).
Host folds 1/sqrt(Dh) into Wq/bq, drops bk (softmax-invariant), splits bo
across the two cores of each batch. Each core computes a transposed partial
output outT [1024, 2048]; host sums core pairs and transposes.

Per-core dataflow (all matmuls float32r = FP22 single-pass, full PE rate):
  phase 1: qT/kT = W^T-chunks^T @ xT-chunks (transposed layout, d-on-partition),
           v natural (seq-on-partition) with a ones column per head (v_aug),
           biases applied as K=1 rank-1 matmuls.
  phase 2: per head: S^T tiles = kT^T @ qT -> PSUM, exp on ACT -> P (SBUF),
           ctx^T[65, q] += v_aug^T @ P (row 64 = softmax denominator l),
           normalize via reciprocal + partition_broadcast + DVE multiply.
  phase 3: outT[e, q] = WoST^T @ ctxT_norm + bo/2 rank-1, evict, DMA out.
"""
import numpy as np
from contextlib import ExitStack

import concourse.bass as bass
import concourse.bacc as bacc
import concourse.mybir as mybir
import concourse.tile as tile
from concourse.bass_utils import run_bass_kernel_spmd

F32 = mybir.dt.float32
F32R = mybir.dt.float32r

B = 4
S = 2048
D = 1024
COLS = 512          # qkv cols per core (8 heads x 64)
NHEAD = 8           # heads per core
DH = 64
N = 512             # matmul moving free dim
DCH = D // 128      # 8 contraction chunks for projections
SC = S // N         # 4 seq chunks of 512
CC = COLS // 128    # 4 col chunks
KC = S // 128       # 16 key chunks
QH = S // 1024      # 2 query halves of 1024

_CACHE = {}


def _build():
    nc = bacc.Bacc("TRN2", target_bir_lowering=False, debug=False, num_devices=8)

    xt = nc.declare_dram_parameter("xt", [D, S], F32R, isOutput=False)
    wqt = nc.declare_dram_parameter("wqt", [D, COLS], F32R, isOutput=False)
    wkt = nc.declare_dram_parameter("wkt", [D, COLS], F32R, isOutput=False)
    wvt = nc.declare_dram_parameter("wvt", [D, COLS], F32R, isOutput=False)
    wot = nc.declare_dram_parameter("wot", [COLS, D], F32R, isOutput=False)
    bq = nc.declare_dram_parameter("bq", [1, COLS], F32R, isOutput=False)
    bv = nc.declare_dram_parameter("bv", [1, COLS], F32R, isOutput=False)
    bo2 = nc.declare_dram_parameter("bo2", [1, D], F32R, isOutput=False)
    out = nc.declare_dram_parameter("out", [D, S], F32, isOutput=True)

    with ExitStack() as ctx:
        tc = ctx.enter_context(tile.TileContext(nc))

        const = ctx.enter_context(tc.tile_pool(name="const", bufs=1))
        ones_f32 = const.tile([128, N], F32, tag="ones_f32")
        nc.vector.memset(ones_f32[:], 1.0)
        ones_t = const.tile([1, N], F32R, tag="ones")
        nc.vector.tensor_copy(out=ones_t[:], in_=ones_f32[0:1, :])
        bq_t = const.tile([1, COLS], F32R, tag="bq")
        nc.sync.dma_start(out=bq_t[:], in_=bq[:])
        bv_t = const.tile([1, COLS], F32R, tag="bv")
        nc.sync.dma_start(out=bv_t[:], in_=bv[:])
        bo_t = const.tile([1, D], F32R, tag="bo")
        nc.sync.dma_start(out=bo_t[:], in_=bo2[:])

        # persistent activations
        qkv = ctx.enter_context(tc.tile_pool(name="qkv", bufs=1))
        qT = [qkv.tile([128, S], F32R, tag=f"qt{c}", name=f"qt{c}") for c in range(CC)]
        kT = [qkv.tile([128, S], F32R, tag=f"kt{c}", name=f"kt{c}") for c in range(CC)]
        # v_aug: per seq chunk of 128 rows, 8 heads x (64 v cols + ones col)
        v_sb = [qkv.tile([128, NHEAD * 65], F32R, tag=f"v{i}", name=f"v{i}") for i in range(KC)]
        ctxn = ctx.enter_context(tc.tile_pool(name="ctxn", bufs=1))
        cn = [ctxn.tile([128, S], F32R, tag=f"cn{c}", name=f"cn{c}") for c in range(CC)]

        for i in range(KC):
            # ones column per head at local col 64
            va = v_sb[i][:].rearrange("p (h c) -> p h c", c=65)
            nc.vector.tensor_copy(
                out=va[:, :, 64:65],
                in_=ones_f32[:, 0:NHEAD].rearrange("p (h c) -> p h c", c=1),
            )

        # ---------------- phase 1: projections ----------------
        with tc.tile_pool(name="ph1ps", bufs=6, space="PSUM") as pp, \
             tc.tile_pool(name="xt", bufs=16) as xtp, \
             tc.tile_pool(name="w", bufs=6) as wp, \
             tc.tile_pool(name="wv", bufs=4) as wvp:
            for sc in range(SC):
                xts = []
                for d in range(DCH):
                    t = xtp.tile([128, N], F32R, tag="xt")
                    nc.sync.dma_start(
                        out=t[:], in_=xt[d * 128:(d + 1) * 128, sc * N:(sc + 1) * N]
                    )
                    xts.append(t)
                # qT, kT: [cols-chunk 128, seq 512] = sum_d W[d,c]^T @ xT[d,s]
                for proj, wsrc, bias in (("q", wqt, bq_t), ("k", wkt, None)):
                    dst = qT if proj == "q" else kT
                    for c in range(CC):
                        ps = pp.tile([128, N], F32, tag="ps")
                        for d in range(DCH):
                            w_t = wp.tile([128, 128], F32R, tag="w")
                            nc.sync.dma_start(
                                out=w_t[:],
                                in_=wsrc[d * 128:(d + 1) * 128, c * 128:(c + 1) * 128],
                            )
                            nc.tensor.matmul(
                                ps[:], w_t[:], xts[d][:],
                                start=(d == 0),
                                stop=(d == DCH - 1 and bias is None),
                            )
                        if bias is not None:
                            nc.tensor.matmul(
                                ps[:], bias[:, c * 128:(c + 1) * 128], ones_t[:],
                                start=False, stop=True,
                            )
                        nc.vector.tensor_copy(
                            out=dst[c][:, sc * N:(sc + 1) * N], in_=ps[:]
                        )
                # v natural: [seq 128, cols 512] = sum_d xT[d, s128]^T @ WvT[d, :]
                vps = [pp.tile([128, N], F32, tag="ps", name=f"vps{m}") for m in range(4)]
                for d in range(DCH):
                    wv_t = wvp.tile([128, N], F32R, tag="wv")
                    nc.sync.dma_start(
                        out=wv_t[:], in_=wvt[d * 128:(d + 1) * 128, :]
                    )
                    for m in range(4):
                        nc.tensor.matmul(
                            vps[m][:],
                            xts[d][:, m * 128:(m + 1) * 128],
                            wv_t[:],
                            start=(d == 0), stop=False,
                        )
                for m in range(4):
                    nc.tensor.matmul(
                        vps[m][:], ones_t[:, 0:128], bv_t[:],
                        start=False, stop=True,
                    )
                    dst = v_sb[sc * 4 + m][:].rearrange("p (h c) -> p h c", c=65)
                    src = vps[m][:].rearrange("p (h c) -> p h c", c=64)
                    nc.vector.tensor_copy(out=dst[:, :, 0:64], in_=src[:])

        # ---------------- phase 2: attention per head ----------------
        with tc.tile_pool(name="stps", bufs=2, space="PSUM") as stp, \
             tc.tile_pool(name="ctxps", bufs=1, space="PSUM") as cxp, \
             tc.tile_pool(name="p", bufs=3) as pb, \
             tc.tile_pool(name="r", bufs=1) as rp, \
             tc.tile_pool(name="rb", bufs=1) as rbp:
            for h in range(NHEAD):
                c = h // 2
                po = (h % 2) * 64
                ctx_ps = cxp.tile([65, S], F32, tag="ctx")
                va = v_sb
                for kc in range(KC):
                    lv = v_sb[kc][:, h * 65:(h + 1) * 65]
                    for qh in range(QH):
                        st = stp.tile([128, 1024], F32, tag="st")
                        for qq in range(2):
                            qs = qh * 1024 + qq * N
                            nc.tensor.matmul(
                                st[:, qq * N:(qq + 1) * N],
                                kT[c][po:po + 64, kc * 128:(kc + 1) * 128],
                                qT[c][po:po + 64, qs:qs + N],
                                start=True, stop=True,
                            )
                        p_t = pb.tile([128, 1024], F32R, tag="p")
                        nc.scalar.activation(
                            p_t[:], st[:], mybir.ActivationFunctionType.Exp
                        )
                        for qq in range(2):
                            qs = qh * 1024 + qq * N
                            nc.tensor.matmul(
                                ctx_ps[0:65, qs:qs + N],
                                lv,
                                p_t[:, qq * N:(qq + 1) * N],
                                start=(kc == 0), stop=(kc == KC - 1),
                            )
                # normalize: rows 0..63 /= row 64
                r_t = rp.tile([1, S], F32, tag="r")
                nc.vector.reciprocal(r_t[:], ctx_ps[64:65, :])
                rb_t = rbp.tile([64, S], F32, tag="rb")
                nc.gpsimd.partition_broadcast(rb_t[:], r_t[:])
                nc.vector.tensor_tensor(
                    out=cn[c][po:po + 64, :],
                    in0=ctx_ps[0:64, :],
                    in1=rb_t[:],
                    op=mybir.AluOpType.mult,
                )

        # ---------------- phase 3: output projection ----------------
        with tc.tile_pool(name="outps", bufs=4, space="PSUM") as op, \
             tc.tile_pool(name="wo", bufs=6) as wop, \
             tc.tile_pool(name="outsb", bufs=4) as osb:
            for e in range(DCH):
                wo_ts = []
                for c2 in range(CC):
                    w_t = wop.tile([128, 128], F32R, tag="wo")
                    nc.sync.dma_start(
                        out=w_t[:],
                        in_=wot[c2 * 128:(c2 + 1) * 128, e * 128:(e + 1) * 128],
                    )
                    wo_ts.append(w_t)
                for qc in range(SC):
                    ps = op.tile([128, N], F32, tag="ops")
                    for c2 in range(CC):
                        nc.tensor.matmul(
                            ps[:], wo_ts[c2][:], cn[c2][:, qc * N:(qc + 1) * N],
                            start=(c2 == 0), stop=False,
                        )
                    nc.tensor.matmul(
                        ps[:], bo_t[:, e * 128:(e + 1) * 128], ones_t[:],
                        start=False, stop=True,
                    )
                    o_t = osb.tile([128, N], F32, tag="osb")
                    nc.vector.tensor_copy(out=o_t[:], in_=ps[:])
                    nc.sync.dma_start(
                        out=out[e * 128:(e + 1) * 128, qc * N:(qc + 1) * N],
                        in_=o_t[:],
                    )

    nc.compile()
    return nc


def _get_nc():
    if "nc" not in _CACHE:
        _CACHE["nc"] = _build()
    return _CACHE["nc"]


def _in_maps(x, Wq, bq, Wk, bk, Wv, bv, Wo, bo):
    maps = []
    for core in range(8):
        b, g = core // 2, core % 2
        cols = slice(g * COLS, (g + 1) * COLS)
        maps.append({
            "xt": np.ascontiguousarray(x[b].T),
            "wqt": np.ascontiguousarray((Wq[cols] / 8.0).T),
            "bq": (bq[cols] / 8.0).reshape(1, COLS).copy(),
            "wkt": np.ascontiguousarray(Wk[cols].T),
            "wvt": np.ascontiguousarray(Wv[cols].T),
            "bv": bv[cols].reshape(1, COLS).copy(),
            "wot": np.ascontiguousarray(Wo[:, cols].T),
            "bo2": (bo / 2.0).reshape(1, D).copy(),
        })
    return maps


def kernel(x, Wq, bq, Wk, bk, Wv, bv, Wo, bo, _trace=False, **trace_kwargs):
    x = np.asarray(x, dtype=np.float32)
    Wq = np.asarray(Wq, dtype=np.float32)
    bq = np.asarray(bq, dtype=np.float32)
    Wk = np.asarray(Wk, dtype=np.float32)
    Wv = np.asarray(Wv, dtype=np.float32)
    bv = np.asarray(bv, dtype=np.float32)
    Wo = np.asarray(Wo, dtype=np.float32)
    bo = np.asarray(bo, dtype=np.float32)

    nc = _get_nc()
    maps = _in_maps(x, Wq, bq, Wk, None, Wv, bv, Wo, bo)
    res = run_bass_kernel_spmd(nc, maps, list(range(8)), trace=_trace, **trace_kwargs)

    outp = np.empty((B, S, D), np.float32)
    for b in range(B):
        t = res.results[2 * b]["out"] + res.results[2 * b + 1]["out"]
        outp[b] = t.T
    if _trace:
        return outp, res
    return outp


# revision 8
# speedup vs baseline: 1.2343x; 1.2343x over previous
"""Multi-head attention (B=4, S=2048, D=1024, H=16, Dh=64) on 8 trn2 cores.

Sharding: core c -> batch b=c//2, head-group g=c%2 (8 heads = 512 qkv cols).
Host folds 1/sqrt(Dh) into Wq/bq, drops bk (softmax-invariant), splits bo
across the two cores of each batch. Each core computes a transposed partial
output outT [1024, 2048]; host sums core pairs and transposes.

Per-core dataflow:
  phase 1: qT/kT = W^T-chunks^T @ xT-chunks (transposed layout, d-on-partition),
           v natural (seq-on-partition) with a ones column per head (v_aug).
           x/W in bf16 (PE full rate); q bias folded into the PSUM eviction as
           a per-partition tensor_scalar add; v bias as a K=1 rank-1 matmul.
  phase 2: per head: S^T tiles = kT^T @ qT -> PSUM, exp on ACT -> P (SBUF bf16),
           ctx^T[65, q] += v_aug^T @ P (row 64 = softmax denominator l),
           normalize via reciprocal_approx_fast + partition_broadcast + DVE mult.
  phase 3: outT[e, q] = WoST^T @ ctxT_norm in float32r (output precision),
           bo/2 folded into eviction, DMA out.
"""
import numpy as np
import ml_dtypes
from contextlib import ExitStack

import concourse.bass as bass
import concourse.bacc as bacc
import concourse.mybir as mybir
import concourse.tile as tile
from concourse.bass_utils import run_bass_kernel_spmd

F32 = mybir.dt.float32
F32R = mybir.dt.float32r
BF16 = mybir.dt.bfloat16
NP_BF16 = ml_dtypes.bfloat16

B = 4
S = 2048
D = 1024
COLS = 512          # qkv cols per core (8 heads x 64)
NHEAD = 8           # heads per core
N = 512             # matmul moving free dim
DCH = D // 128      # 8 contraction chunks for projections
SC = S // N         # 4 seq chunks of 512
CC = COLS // 128    # 4 col chunks
KC = S // 128       # 16 key chunks
QH = S // 1024      # 2 query halves of 1024

_CACHE = {}


def _build():
    nc = bacc.Bacc("TRN2", target_bir_lowering=False, debug=False, num_devices=8)

    xt = nc.declare_dram_parameter("xt", [D, S], BF16, isOutput=False)
    wqt = nc.declare_dram_parameter("wqt", [D, COLS], BF16, isOutput=False)
    wkt = nc.declare_dram_parameter("wkt", [D, COLS], BF16, isOutput=False)
    wvt = nc.declare_dram_parameter("wvt", [D, COLS], BF16, isOutput=False)
    wot = nc.declare_dram_parameter("wot", [COLS, D], F32R, isOutput=False)
    bq = nc.declare_dram_parameter("bq", [128, CC], F32, isOutput=False)
    bv = nc.declare_dram_parameter("bv", [1, COLS], F32R, isOutput=False)
    bo2 = nc.declare_dram_parameter("bo2", [128, DCH], F32, isOutput=False)
    out = nc.declare_dram_parameter("out", [D, S], F32, isOutput=True)

    with ExitStack() as ctx:
        tc = ctx.enter_context(tile.TileContext(nc))

        const = ctx.enter_context(tc.tile_pool(name="const", bufs=1))
        ones_f32 = const.tile([128, 128], F32, tag="ones_f32")
        nc.vector.memset(ones_f32[:], 1.0)
        ones_r = const.tile([1, 128], F32R, tag="ones_r")
        nc.vector.tensor_copy(out=ones_r[:], in_=ones_f32[0:1, :])
        bq_t = const.tile([128, CC], F32, tag="bq")
        nc.sync.dma_start(out=bq_t[:], in_=bq[:])
        bv_t = const.tile([1, COLS], F32R, tag="bv")
        nc.sync.dma_start(out=bv_t[:], in_=bv[:])
        bo_t = const.tile([128, DCH], F32, tag="bo")
        nc.sync.dma_start(out=bo_t[:], in_=bo2[:])

        # persistent activations
        qkv = ctx.enter_context(tc.tile_pool(name="qkv", bufs=1))
        qT = [qkv.tile([128, S], BF16, tag=f"qt{c}", name=f"qt{c}") for c in range(CC)]
        kT = [qkv.tile([128, S], BF16, tag=f"kt{c}", name=f"kt{c}") for c in range(CC)]
        # v_aug: per seq chunk of 128 rows, 8 heads x (64 v cols + ones col)
        v_sb = [qkv.tile([128, NHEAD * 65], BF16, tag=f"v{i}", name=f"v{i}")
                for i in range(KC)]
        ctxn = ctx.enter_context(tc.tile_pool(name="ctxn", bufs=1))
        cn = [ctxn.tile([128, S], F32R, tag=f"cn{c}", name=f"cn{c}") for c in range(CC)]

        for i in range(KC):
            # ones column per head at local col 64
            va = v_sb[i][:].rearrange("p (h c) -> p h c", c=65)
            nc.vector.tensor_copy(
                out=va[:, :, 64:65],
                in_=ones_f32[:, 0:NHEAD].rearrange("p (h c) -> p h c", c=1),
            )

        # ---------------- phase 1: projections ----------------
        with tc.tile_pool(name="ph1ps", bufs=6, space="PSUM") as pp, \
             tc.tile_pool(name="xtp", bufs=16) as xtp, \
             tc.tile_pool(name="w", bufs=6) as wp, \
             tc.tile_pool(name="wv", bufs=4) as wvp:
            for sc in range(SC):
                xts = []
                for d in range(DCH):
                    t = xtp.tile([128, N], BF16, tag="xt", name=f"xt{sc}_{d}")
                    nc.sync.dma_start(
                        out=t[:], in_=xt[d * 128:(d + 1) * 128, sc * N:(sc + 1) * N]
                    )
                    xts.append(t)
                # qT, kT: [cols-chunk 128, seq 512] = sum_d W[d,c]^T @ xT[d,s]
                for proj, wsrc, bias in (("q", wqt, bq_t), ("k", wkt, None)):
                    dst = qT if proj == "q" else kT
                    for c in range(CC):
                        ps = pp.tile([128, N], F32, tag="ps", name=f"ps{proj}{sc}_{c}")
                        for d in range(DCH):
                            w_t = wp.tile([128, 128], BF16, tag="w",
                                          name=f"w{proj}{sc}_{c}_{d}")
                            nc.sync.dma_start(
                                out=w_t[:],
                                in_=wsrc[d * 128:(d + 1) * 128, c * 128:(c + 1) * 128],
                            )
                            nc.tensor.matmul(
                                ps[:], w_t[:], xts[d][:],
                                start=(d == 0), stop=(d == DCH - 1),
                            )
                        if bias is not None:
                            nc.vector.tensor_scalar_add(
                                out=dst[c][:, sc * N:(sc + 1) * N],
                                in0=ps[:],
                                scalar1=bias[:, c:c + 1],
                            )
                        else:
                            nc.vector.tensor_copy(
                                out=dst[c][:, sc * N:(sc + 1) * N], in_=ps[:]
                            )
                # v natural: [seq 128, cols 512] = sum_d xT[d, s128]^T @ WvT[d, :]
                vps = [pp.tile([128, N], F32, tag="ps", name=f"vps{sc}_{m}")
                       for m in range(4)]
                for d in range(DCH):
                    wv_t = wvp.tile([128, N], BF16, tag="wv", name=f"wv{sc}_{d}")
                    nc.sync.dma_start(
                        out=wv_t[:], in_=wvt[d * 128:(d + 1) * 128, :]
                    )
                    for m in range(4):
                        nc.tensor.matmul(
                            vps[m][:],
                            xts[d][:, m * 128:(m + 1) * 128],
                            wv_t[:],
                            start=(d == 0), stop=False,
                        )
                for m in range(4):
                    nc.tensor.matmul(
                        vps[m][:], ones_r[:], bv_t[:],
                        start=False, stop=True,
                    )
                    dst = v_sb[sc * 4 + m][:].rearrange("p (h c) -> p h c", c=65)
                    src = vps[m][:].rearrange("p (h c) -> p h c", c=64)
                    nc.vector.tensor_copy(out=dst[:, :, 0:64], in_=src[:])

        # ---------------- phase 2: attention per head ----------------
        with tc.tile_pool(name="stps", bufs=2, space="PSUM") as stp, \
             tc.tile_pool(name="ctxps", bufs=1, space="PSUM") as cxp, \
             tc.tile_pool(name="p", bufs=3) as pb, \
             tc.tile_pool(name="r", bufs=1) as rp, \
             tc.tile_pool(name="rb", bufs=1) as rbp:
            for h in range(NHEAD):
                c = h // 2
                po = (h % 2) * 64
                ctx_ps = cxp.tile([65, S], F32, tag="ctx", name=f"ctx{h}")
                for kc in range(KC):
                    lv = v_sb[kc][:, h * 65:(h + 1) * 65]
                    for qh in range(QH):
                        st = stp.tile([128, 1024], F32, tag="st", name=f"st{h}_{kc}_{qh}")
                        for qq in range(2):
                            qs = qh * 1024 + qq * N
                            nc.tensor.matmul(
                                st[:, qq * N:(qq + 1) * N],
                                kT[c][po:po + 64, kc * 128:(kc + 1) * 128],
                                qT[c][po:po + 64, qs:qs + N],
                                start=True, stop=True,
                            )
                        p_t = pb.tile([128, 1024], BF16, tag="p", name=f"p{h}_{kc}_{qh}")
                        nc.scalar.activation(
                            p_t[:], st[:], mybir.ActivationFunctionType.Exp
                        )
                        for qq in range(2):
                            qs = qh * 1024 + qq * N
                            nc.tensor.matmul(
                                ctx_ps[0:65, qs:qs + N],
                                lv,
                                p_t[:, qq * N:(qq + 1) * N],
                                start=(kc == 0), stop=(kc == KC - 1),
                            )
                # normalize: rows 0..63 /= row 64
                l_sb = rp.tile([1, S], F32, tag="l", name=f"l{h}")
                nc.vector.tensor_copy(out=l_sb[:], in_=ctx_ps[64:65, :])
                r_t = rp.tile([1, S], F32, tag="r", name=f"r{h}")
                nc.vector.reciprocal_approx_fast(r_t[:], l_sb[:])
                rb_t = rbp.tile([64, S], F32, tag="rb", name=f"rb{h}")
                nc.gpsimd.partition_broadcast(rb_t[:], r_t[:])
                nc.vector.tensor_tensor(
                    out=cn[c][po:po + 64, :],
                    in0=ctx_ps[0:64, :],
                    in1=rb_t[:],
                    op=mybir.AluOpType.mult,
                )

        # ---------------- phase 3: output projection ----------------
        with tc.tile_pool(name="outps", bufs=4, space="PSUM") as op, \
             tc.tile_pool(name="wo", bufs=6) as wop, \
             tc.tile_pool(name="outsb", bufs=4) as osb:
            for e in range(DCH):
                wo_ts = []
                for c2 in range(CC):
                    w_t = wop.tile([128, 128], F32R, tag="wo", name=f"wo{e}_{c2}")
                    nc.sync.dma_start(
                        out=w_t[:],
                        in_=wot[c2 * 128:(c2 + 1) * 128, e * 128:(e + 1) * 128],
                    )
                    wo_ts.append(w_t)
                for qc in range(SC):
                    ps = op.tile([128, N], F32, tag="ops", name=f"ops{e}_{qc}")
                    for c2 in range(CC):
                        nc.tensor.matmul(
                            ps[:], wo_ts[c2][:], cn[c2][:, qc * N:(qc + 1) * N],
                            start=(c2 == 0), stop=(c2 == CC - 1),
                        )
                    o_t = osb.tile([128, N], F32, tag="osb", name=f"osb{e}_{qc}")
                    nc.vector.tensor_scalar_add(
                        out=o_t[:], in0=ps[:], scalar1=bo_t[:, e:e + 1],
                    )
                    nc.sync.dma_start(
                        out=out[e * 128:(e + 1) * 128, qc * N:(qc + 1) * N],
                        in_=o_t[:],
                    )

    nc.compile()
    return nc


def _get_nc():
    if "nc" not in _CACHE:
        _CACHE["nc"] = _build()
    return _CACHE["nc"]


def _in_maps(x, Wq, bq, Wk, Wv, bv, Wo, bo):
    maps = []
    for core in range(8):
        b, g = core // 2, core % 2
        cols = slice(g * COLS, (g + 1) * COLS)
        maps.append({
            "xt": np.ascontiguousarray(x[b].T).astype(NP_BF16),
            "wqt": np.ascontiguousarray((Wq[cols] / 8.0).T).astype(NP_BF16),
            "bq": np.ascontiguousarray((bq[cols] / 8.0).reshape(CC, 128).T),
            "wkt": np.ascontiguousarray(Wk[cols].T).astype(NP_BF16),
            "wvt": np.ascontiguousarray(Wv[cols].T).astype(NP_BF16),
            "bv": bv[cols].reshape(1, COLS).copy(),
            "wot": np.ascontiguousarray(Wo[:, cols].T),
            "bo2": np.ascontiguousarray((bo / 2.0).reshape(DCH, 128).T),
        })
    return maps


def kernel(x, Wq, bq, Wk, bk, Wv, bv, Wo, bo, _trace=False, **trace_kwargs):
    x = np.asarray(x, dtype=np.float32)
    Wq = np.asarray(Wq, dtype=np.float32)
    bq = np.asarray(bq, dtype=np.float32)
    Wk = np.asarray(Wk, dtype=np.float32)
    Wv = np.asarray(Wv, dtype=np.float32)
    bv = np.asarray(bv, dtype=np.float32)
    Wo = np.asarray(Wo, dtype=np.float32)
    bo = np.asarray(bo, dtype=np.float32)

    nc = _get_nc()
    maps = _in_maps(x, Wq, bq, Wk, Wv, bv, Wo, bo)
    res = run_bass_kernel_spmd(nc, maps, list(range(8)), trace=_trace, **trace_kwargs)

    outp = np.empty((B, S, D), np.float32)
    for b in range(B):
        t = res.results[2 * b]["out"] + res.results[2 * b + 1]["out"]
        outp[b] = t.T
    if _trace:
        return outp, res
    return outp
